# revision 2
# baseline (speedup 1.0000x reference)
"""Trainium2 Bass kernel for the 4-layer sum/product circuit
(nn_KnowledgeLayer): h = enc(x); h = h[idx0].prod(1); h = h[idx1].sum(1);
h = h[idx2].prod(1); h = h[idx3].sum(1).

Strategy v2 (per core, batch sharded 8 x 128 columns):
  * Batch lives on the 128 SBUF PARTITIONS; the 4098-row enc table lives
    along the free dimension as enc[128 batch, 4098 feat] (x^T | 1-x^T | 0|1),
    built on-chip with 16 PE transposes.
  * Host composes the four index maps into TWO flat operand streams
    (A, B) of 32768 int16 indices each, ordered so that within each chunk
    of 4096 slots the layer reductions are contiguous half-splits.
  * Device: gpsimd ap_gather (SBUF->SBUF, free-dim gather, ~1 elem/cycle)
    pulls both streams chunk by chunk; DVE does mul / add / mul / add on
    contiguous halves; PE transposes h3 back to row-major for the output.

The bass program is identical for all 8 cores (pure SPMD); per-core data
(x batch slice) is supplied via in_maps. Indices are runtime inputs.
"""

import numpy as np

N_VARS = 2048
BATCH = 1024
NCORES = 8
BSLICE = BATCH // NCORES          # 128
TABLE = 2 * N_VARS + 2            # 4098
NIDX = 32768                      # h0 slots
NOUT = 4096                       # h3 rows

NCHUNK = 8
CH = NIDX // NCHUNK               # 4096 h0 slots per chunk
OUTC = NOUT // NCHUNK             # 512 h3 outputs per chunk


# ----------------------------------------------------------------------------
# host-side index preparation
# ----------------------------------------------------------------------------

def _remap(e):
    """reference enc row -> our table column.
    table: [0,2048) = x[f], [2048,4096) = 1-x[f], 4096 = 0, 4097 = 1."""
    out = np.empty_like(e)
    out[e == 0] = 2 * N_VARS
    out[e == 1] = 2 * N_VARS + 1
    even = (e >= 2) & (e % 2 == 0)
    out[even] = (e[even] - 2) // 2
    odd = (e >= 3) & (e % 2 == 1)
    out[odd] = N_VARS + (e[odd] - 3) // 2
    return out


def _compose_indices(idx0, idx1, idx2, idx3):
    J = idx3.reshape(-1)              # [8192]  (i, a)   layer3 sum pairs
    K = idx2[J].reshape(-1)           # [16384] (i, a, b) layer2 prod pairs
    L = idx1[K].reshape(-1)           # [32768] (i, a, b, c) layer1 sum pairs
    AB = idx0[L]                      # [32768, 2]       layer0 prod pairs
    A = _remap(AB[:, 0].astype(np.int64))
    B = _remap(AB[:, 1].astype(np.int64))

    # Reorder from (i,a,b,c) nesting to per-chunk contiguous-half layout:
    # chunk c0 handles i in [c0*OUTC, (c0+1)*OUTC); within the chunk the
    # slot order is (c, b, a, ii) so each reduction is a half-split:
    #   h1 = h0[:2048]+h0[2048:], h2 = h1[:1024]*h1[1024:], h3 = h2[:512]+h2[512:]
    def reorder(S):
        S = S.reshape(NCHUNK, OUTC, 2, 2, 2)     # [c0, ii, a, b, c]
        S = S.transpose(0, 4, 3, 2, 1)           # [c0, c, b, a, ii]
        return np.ascontiguousarray(S).reshape(-1)

    return reorder(A), reorder(B)


def _wrap_idx(stream_idx):
    """ap_gather index layout: each 16-partition group holds its core's
    indices wrapped p = s % 16, col = s // 16; replicated for 8 cores."""
    w = stream_idx.reshape(-1, 16).T.astype(np.int16)   # [16, NIDX//16]
    return np.ascontiguousarray(np.tile(w, (8, 1)))     # [128, NIDX//16]


# ----------------------------------------------------------------------------
# bass program (built once, cached)
# ----------------------------------------------------------------------------

_CACHED = {}


def _build_program():
    import concourse.bacc as bacc
    import concourse.mybir as mybir
    from concourse.tile import TileContext

    f32 = mybir.dt.float32
    i16 = mybir.dt.int16

    nc = bacc.Bacc("TRN2", target_bir_lowering=False, debug=False)

    xs = nc.dram_tensor("xs", [N_VARS, BSLICE], f32, kind="ExternalInput")
    idxa = nc.dram_tensor("idxa", [128, NIDX // 16], i16, kind="ExternalInput")
    idxb = nc.dram_tensor("idxb", [128, NIDX // 16], i16, kind="ExternalInput")
    idn = nc.dram_tensor("idn", [128, 128], f32, kind="ExternalInput")
    out = nc.dram_tensor("out", [NOUT, BSLICE], f32, kind="ExternalOutput")

    with TileContext(nc) as tc:
        with tc.tile_pool(name="setup", bufs=1) as sp, \
             tc.tile_pool(name="gather", bufs=2) as gp, \
             tc.tile_pool(name="mid", bufs=2) as mp, \
             tc.tile_pool(name="tpsum", bufs=2, space="PSUM") as pp, \
             tc.tile_pool(name="opsum", bufs=4, space="PSUM") as op_pool, \
             tc.tile_pool(name="outp", bufs=2) as outp:

            ident = sp.tile([128, 128], f32, tag="ident")
            nc.sync.dma_start(out=ident[:, :], in_=idn[:, :])

            ia = sp.tile([128, NIDX // 16], i16, tag="ia")
            ib = sp.tile([128, NIDX // 16], i16, tag="ib")
            nc.sync.dma_start(out=ia[:, :], in_=idxa[:, :])
            nc.sync.dma_start(out=ib[:, :], in_=idxb[:, :])

            # ---- build enc table: [128 batch, 4098] = [x^T | 1-x^T | 0 | 1]
            xt = sp.tile([128, 16, BSLICE], f32, tag="xt")
            nc.sync.dma_start(
                out=xt[:, :, :],
                in_=xs.rearrange("(t p) f -> p t f", p=128))
            enc = sp.tile([128, TABLE], f32, tag="enc")
            for t in range(16):
                ps = pp.tile([128, 128], f32, tag="ps")
                nc.tensor.transpose(ps[:, :], xt[:, t, :], ident[:, :])
                nc.scalar.copy(enc[:, t * 128:(t + 1) * 128], ps[:, :])
            # 1 - x  ==  (x * -1) + 1
            nc.vector.tensor_scalar(
                out=enc[:, N_VARS:2 * N_VARS], in0=enc[:, 0:N_VARS],
                scalar1=-1.0, scalar2=1.0,
                op0=mybir.AluOpType.mult, op1=mybir.AluOpType.add)
            nc.vector.memset(enc[:, 2 * N_VARS:2 * N_VARS + 1], 0.0)
            nc.vector.memset(enc[:, 2 * N_VARS + 1:2 * N_VARS + 2], 1.0)

            # ---- main chunk loop ----
            ccols = CH // 16          # idx columns per chunk = 256
            for c in range(NCHUNK):
                ga = gp.tile([128, CH], f32, tag="ga")
                gb = gp.tile([128, CH], f32, tag="gb")
                nc.gpsimd.ap_gather(
                    out_ap=ga[:, :], in_ap=enc[:, :],
                    idxs_ap=ia[:, c * ccols:(c + 1) * ccols],
                    channels=128, num_elems=TABLE, d=1, num_idxs=CH)
                nc.gpsimd.ap_gather(
                    out_ap=gb[:, :], in_ap=enc[:, :],
                    idxs_ap=ib[:, c * ccols:(c + 1) * ccols],
                    channels=128, num_elems=TABLE, d=1, num_idxs=CH)

                h0 = mp.tile([128, CH], f32, tag="h0")
                nc.vector.tensor_mul(h0[:, :], ga[:, :], gb[:, :])
                h1 = mp.tile([128, CH // 2], f32, tag="h1")
                nc.vector.tensor_add(h1[:, :], h0[:, :CH // 2], h0[:, CH // 2:])
                h2 = mp.tile([128, CH // 4], f32, tag="h2")
                nc.vector.tensor_mul(h2[:, :], h1[:, :CH // 4], h1[:, CH // 4:])
                h3 = mp.tile([128, OUTC], f32, tag="h3")
                nc.vector.tensor_add(h3[:, :], h2[:, :OUTC], h2[:, OUTC:])

                # transpose h3 [128 batch, 512] -> out rows [512, 128]
                ot = outp.tile([128, 4, 128], f32, tag="ot")
                for t in range(4):
                    pso = op_pool.tile([128, 128], f32, tag="pso")
                    nc.tensor.transpose(
                        pso[:, :], h3[:, t * 128:(t + 1) * 128], ident[:, :])
                    nc.scalar.copy(ot[:, t, :], pso[:, :])
                nc.sync.dma_start(
                    out=out[c * OUTC:(c + 1) * OUTC, :]
                        .rearrange("(t p) f -> p t f", p=128),
                    in_=ot[:, :, :])

    nc.compile()
    return nc


def _get_program():
    if "nc" not in _CACHED:
        _CACHED["nc"] = _build_program()
    return _CACHED["nc"]


# ----------------------------------------------------------------------------
# public entry point
# ----------------------------------------------------------------------------

def kernel(x, idx0, idx1, idx2, idx3, _trace=False, _trace_kwargs=None):
    from concourse.bass_utils import run_bass_kernel_spmd

    x = np.ascontiguousarray(np.asarray(x, dtype=np.float32))
    sA, sB = _compose_indices(
        np.asarray(idx0), np.asarray(idx1), np.asarray(idx2), np.asarray(idx3))
    wa, wb = _wrap_idx(sA), _wrap_idx(sB)
    idn = np.eye(128, dtype=np.float32)

    nc = _get_program()
    in_maps = []
    for c in range(NCORES):
        xs = np.ascontiguousarray(x[:, c * BSLICE:(c + 1) * BSLICE])
        in_maps.append({"xs": xs, "idxa": wa, "idxb": wb, "idn": idn})

    kwargs = {}
    if _trace:
        kwargs["trace"] = True
        if _trace_kwargs:
            kwargs.update(_trace_kwargs)
    res = run_bass_kernel_spmd(nc, in_maps, core_ids=list(range(NCORES)), **kwargs)
    outs = [res.results[c]["out"] for c in range(NCORES)]
    full = np.concatenate(outs, axis=1)
    if _trace:
        kernel.last_exec_time_ns = res.exec_time_ns
        kernel.last_profile = res.profile_json
    return full


# revision 4
# speedup vs baseline: 12.6522x; 12.6522x over previous
"""Trainium2 Bass kernel for the 4-layer sum/product circuit
(nn_KnowledgeLayer): h = enc(x); h = h[idx0].prod(1); h = h[idx1].sum(1);
h = h[idx2].prod(1); h = h[idx3].sum(1).

Strategy v4 (shard the COMPOSED SLOT STREAM, not the batch):
  * Host composes the four index maps into TWO flat operand streams of
    32768 row-indices each into a 4098-row full-batch enc table
    ([x | 1-x | 0 | 1], built host-side as [4098, 1024] f32).
  * Core c owns h3 rows [c*512, (c+1)*512) and gathers FULL 4KB rows
    (all 1024 batch cols) for its slot subtree: 8192 gathers/core of
    4KB instead of 65536 of 512B -- 8x fewer descriptors.
  * dma_gather (SWDGE) calls round-robin over 4 queues so transfers
    overlap; DVE reduces (mul/add/mul) within partitions; the final
    sum pairs adjacent PARTITIONS via a PE matmul with a [128,64]
    pairing matrix; ACT drains PSUM; DMA writes 64 output rows/chunk.

The bass program is identical for all 8 cores (pure SPMD); per-core
index streams differ via in_maps.
"""

import numpy as np

N_VARS = 2048
BATCH = 1024
NCORES = 8
TABLE = 2 * N_VARS + 2            # 4098
NOUT = 4096                       # h3 rows total
CORE_OUT = NOUT // NCORES         # 512 h3 rows per core
NCHUNK = 8
CHO = CORE_OUT // NCHUNK          # 64 h3 rows per chunk
CHS = CHO * 8                     # 512 h0 slots per chunk
NIDXC = CORE_OUT * 8              # 4096 h0 slots per core


# ----------------------------------------------------------------------------
# host-side index preparation
# ----------------------------------------------------------------------------

def _remap(e):
    """reference enc row -> our table row.
    table: [0,2048) = x[f], [2048,4096) = 1-x[f], 4096 = 0, 4097 = 1."""
    out = np.empty_like(e)
    out[e == 0] = 2 * N_VARS
    out[e == 1] = 2 * N_VARS + 1
    even = (e >= 2) & (e % 2 == 0)
    out[even] = (e[even] - 2) // 2
    odd = (e >= 3) & (e % 2 == 1)
    out[odd] = N_VARS + (e[odd] - 3) // 2
    return out


def _compose_indices(idx0, idx1, idx2, idx3):
    J = idx3.reshape(-1)              # [8192]  (i, a)   layer3 sum pairs
    K = idx2[J].reshape(-1)           # [16384] (i, a, b) layer2 prod pairs
    L = idx1[K].reshape(-1)           # [32768] (i, a, b, c) layer1 sum pairs
    AB = idx0[L]                      # [32768, 2]       layer0 prod pairs
    A = _remap(AB[:, 0].astype(np.int64))
    B = _remap(AB[:, 1].astype(np.int64))
    return A.reshape(NOUT, 2, 2, 2), B.reshape(NOUT, 2, 2, 2)


def _core_wrap(S, c):
    """Per-core chunked+wrapped int16 index tensor [128, NCHUNK*CHS//16].

    Chunk k covers i = c*512 + k*64 + ii.  Gather position within a call:
    g = j*128 + p with free block j = cbit*2 + b and partition p = ii*2+a,
    so h1 = h0[:, :2]+h0[:, 2:], h2 = h1[:, :1]*h1[:, 1:2], and the final
    a-sum pairs adjacent partitions (PE matmul).
    SWDGE wraps each call's g-stream: idx[p16, s] = call[s*16 + p16].
    """
    Sc = S[c * CORE_OUT:(c + 1) * CORE_OUT]              # [512, 2, 2, 2]
    Sc = Sc.reshape(NCHUNK, CHO, 2, 2, 2)                # [k, ii, a, b, cb]
    Sc = Sc.transpose(0, 4, 3, 1, 2)                     # [k, cb, b, ii, a]
    calls = Sc.reshape(NCHUNK, CHS)                      # g = ((cb*2+b)*64+ii)*2+a
    w = calls.reshape(NCHUNK, CHS // 16, 16)             # [k, s, p16]
    w = w.transpose(2, 0, 1).astype(np.int16)            # [16, k, s]
    w = w.reshape(16, NCHUNK * (CHS // 16))
    return np.ascontiguousarray(np.tile(w, (8, 1)))      # [128, k*32]


# ----------------------------------------------------------------------------
# bass program (built once, cached)
# ----------------------------------------------------------------------------

_CACHED = {}


def _build_program():
    import concourse.bacc as bacc
    import concourse.mybir as mybir
    from concourse.tile import TileContext

    f32 = mybir.dt.float32
    i16 = mybir.dt.int16

    nc = bacc.Bacc("TRN2", target_bir_lowering=False, debug=False,
                   num_swdge_queues=4)

    enc = nc.dram_tensor("enc", [TABLE, BATCH], f32, kind="ExternalInput")
    idxa = nc.dram_tensor("idxa", [128, NCHUNK * CHS // 16], i16,
                          kind="ExternalInput")
    idxb = nc.dram_tensor("idxb", [128, NCHUNK * CHS // 16], i16,
                          kind="ExternalInput")
    pairs = nc.dram_tensor("pairs", [128, CHO], f32, kind="ExternalInput")
    out = nc.dram_tensor("out", [CORE_OUT, BATCH], f32, kind="ExternalOutput")

    with TileContext(nc) as tc:
        with tc.tile_pool(name="setup", bufs=1) as sp, \
             tc.tile_pool(name="gather", bufs=3) as gp, \
             tc.tile_pool(name="mid", bufs=2) as mp, \
             tc.tile_pool(name="hpsum", bufs=2, space="PSUM") as pp, \
             tc.tile_pool(name="outp", bufs=2) as outp:

            ia = sp.tile([128, NCHUNK * CHS // 16], i16, tag="ia")
            ib = sp.tile([128, NCHUNK * CHS // 16], i16, tag="ib")
            pr = sp.tile([128, CHO], f32, tag="pr")
            nc.sync.dma_start(out=ia[:, :], in_=idxa[:, :])
            nc.sync.dma_start(out=ib[:, :], in_=idxb[:, :])
            nc.sync.dma_start(out=pr[:, :], in_=pairs[:, :])

            ccols = CHS // 16        # 32 idx columns per chunk
            for k in range(NCHUNK):
                ga = gp.tile([128, 4, BATCH], f32, tag="ga")
                gb = gp.tile([128, 4, BATCH], f32, tag="gb")
                nc.gpsimd.dma_gather(
                    out_ap=ga[:, :, :], in_ap=enc[:, :],
                    idxs_ap=ia[:, k * ccols:(k + 1) * ccols],
                    num_idxs=CHS, num_idxs_reg=CHS,
                    elem_size=BATCH, queue_num=(2 * k) % 4)
                nc.gpsimd.dma_gather(
                    out_ap=gb[:, :, :], in_ap=enc[:, :],
                    idxs_ap=ib[:, k * ccols:(k + 1) * ccols],
                    num_idxs=CHS, num_idxs_reg=CHS,
                    elem_size=BATCH, queue_num=(2 * k + 1) % 4)

                h0 = mp.tile([128, 4, BATCH], f32, tag="h0")
                nc.vector.tensor_mul(h0[:, :, :], ga[:, :, :], gb[:, :, :])
                h1 = mp.tile([128, 2, BATCH], f32, tag="h1")
                nc.vector.tensor_add(
                    h1[:, :, :], h0[:, 0:2, :], h0[:, 2:4, :])
                h2 = mp.tile([128, 1, BATCH], f32, tag="h2")
                nc.vector.tensor_mul(
                    h2[:, :, :], h1[:, 0:1, :], h1[:, 1:2, :])

                # final sum pairs adjacent partitions: [128, 1024] -> [64, 1024]
                ps = pp.tile([CHO, BATCH], f32, tag="ps")
                for half in range(2):
                    nc.tensor.matmul(
                        ps[:, half * 512:(half + 1) * 512],
                        lhsT=pr[:, :],
                        rhs=h2[:, 0, half * 512:(half + 1) * 512],
                        start=True, stop=True)
                ot = outp.tile([CHO, BATCH], f32, tag="ot")
                nc.scalar.copy(ot[:, :], ps[:, :])
                nc.sync.dma_start(
                    out=out[k * CHO:(k + 1) * CHO, :], in_=ot[:, :])

    nc.compile()
    return nc


def _get_program():
    if "nc" not in _CACHED:
        _CACHED["nc"] = _build_program()
    return _CACHED["nc"]


# ----------------------------------------------------------------------------
# public entry point
# ----------------------------------------------------------------------------

def kernel(x, idx0, idx1, idx2, idx3, _trace=False, _trace_kwargs=None):
    from concourse.bass_utils import run_bass_kernel_spmd

    x = np.ascontiguousarray(np.asarray(x, dtype=np.float32))
    A, B = _compose_indices(
        np.asarray(idx0), np.asarray(idx1), np.asarray(idx2), np.asarray(idx3))

    enc = np.concatenate(
        [x, 1.0 - x,
         np.zeros((1, BATCH), np.float32),
         np.ones((1, BATCH), np.float32)], axis=0)
    enc = np.ascontiguousarray(enc)

    pairs = np.zeros((128, CHO), np.float32)
    pairs[np.arange(128), np.arange(128) // 2] = 1.0

    nc = _get_program()
    in_maps = []
    for c in range(NCORES):
        in_maps.append({"enc": enc,
                        "idxa": _core_wrap(A, c), "idxb": _core_wrap(B, c),
                        "pairs": pairs})

    kwargs = {}
    if _trace:
        kwargs["trace"] = True
        if _trace_kwargs:
            kwargs.update(_trace_kwargs)
    res = run_bass_kernel_spmd(nc, in_maps, core_ids=list(range(NCORES)), **kwargs)
    outs = [res.results[c]["out"] for c in range(NCORES)]
    full = np.concatenate(outs, axis=0)
    if _trace:
        kernel.last_exec_time_ns = res.exec_time_ns
        kernel.last_profile = res.profile_json
    return full


# revision 5
# speedup vs baseline: 23.0256x; 1.8199x over previous
"""Trainium2 Bass kernel for the 4-layer sum/product circuit
(nn_KnowledgeLayer): h = enc(x); h = h[idx0].prod(1); h = h[idx1].sum(1);
h = h[idx2].prod(1); h = h[idx3].sum(1).

Strategy v4 (shard the COMPOSED SLOT STREAM, not the batch):
  * Host composes the four index maps into TWO flat operand streams of
    32768 row-indices each into a 4098-row full-batch enc table
    ([x | 1-x | 0 | 1], built host-side as [4098, 1024] f32).
  * Core c owns h3 rows [c*512, (c+1)*512) and gathers FULL 4KB rows
    (all 1024 batch cols) for its slot subtree: 8192 gathers/core of
    4KB instead of 65536 of 512B -- 8x fewer descriptors.
  * dma_gather (SWDGE) calls round-robin over 4 queues so transfers
    overlap; DVE reduces (mul/add/mul) within partitions; the final
    sum pairs adjacent PARTITIONS via a PE matmul with a [128,64]
    pairing matrix; ACT drains PSUM; DMA writes 64 output rows/chunk.

The bass program is identical for all 8 cores (pure SPMD); per-core
index streams differ via in_maps.
"""

import numpy as np

N_VARS = 2048
BATCH = 1024
NCORES = 8
TABLE = 2 * N_VARS + 2            # 4098
NOUT = 4096                       # h3 rows total
CORE_OUT = NOUT // NCORES         # 512 h3 rows per core
NCHUNK = 8
CHO = CORE_OUT // NCHUNK          # 64 h3 rows per chunk
CHS = CHO * 8                     # 512 h0 slots per chunk
NIDXC = CORE_OUT * 8              # 4096 h0 slots per core


# ----------------------------------------------------------------------------
# host-side index preparation
# ----------------------------------------------------------------------------

def _remap(e):
    """reference enc row -> our table row.
    table: [0,2048) = x[f], [2048,4096) = 1-x[f], 4096 = 0, 4097 = 1."""
    out = np.empty_like(e)
    out[e == 0] = 2 * N_VARS
    out[e == 1] = 2 * N_VARS + 1
    even = (e >= 2) & (e % 2 == 0)
    out[even] = (e[even] - 2) // 2
    odd = (e >= 3) & (e % 2 == 1)
    out[odd] = N_VARS + (e[odd] - 3) // 2
    return out


def _compose_indices(idx0, idx1, idx2, idx3):
    J = idx3.reshape(-1)              # [8192]  (i, a)   layer3 sum pairs
    K = idx2[J].reshape(-1)           # [16384] (i, a, b) layer2 prod pairs
    L = idx1[K].reshape(-1)           # [32768] (i, a, b, c) layer1 sum pairs
    AB = idx0[L]                      # [32768, 2]       layer0 prod pairs
    A = _remap(AB[:, 0].astype(np.int64))
    B = _remap(AB[:, 1].astype(np.int64))
    return A.reshape(NOUT, 2, 2, 2), B.reshape(NOUT, 2, 2, 2)


def _core_wrap(S, c):
    """Per-core chunked+wrapped int16 index tensor [128, NCHUNK*CHS//16].

    Chunk k covers i = c*512 + k*64 + ii.  Gather position within a call:
    g = j*128 + p with free block j = cbit*2 + b and partition p = ii*2+a,
    so h1 = h0[:, :2]+h0[:, 2:], h2 = h1[:, :1]*h1[:, 1:2], and the final
    a-sum pairs adjacent partitions (PE matmul).
    SWDGE wraps each call's g-stream: idx[p16, s] = call[s*16 + p16].
    """
    Sc = S[c * CORE_OUT:(c + 1) * CORE_OUT]              # [512, 2, 2, 2]
    Sc = Sc.reshape(NCHUNK, CHO, 2, 2, 2)                # [k, ii, a, b, cb]
    Sc = Sc.transpose(0, 4, 3, 1, 2)                     # [k, cb, b, ii, a]
    calls = Sc.reshape(NCHUNK, CHS)                      # g = ((cb*2+b)*64+ii)*2+a
    w = calls.reshape(NCHUNK, CHS // 16, 16)             # [k, s, p16]
    w = w.transpose(2, 0, 1).astype(np.int16)            # [16, k, s]
    w = w.reshape(16, NCHUNK * (CHS // 16))
    return np.ascontiguousarray(np.tile(w, (8, 1)))      # [128, k*32]


# ----------------------------------------------------------------------------
# bass program (built once, cached)
# ----------------------------------------------------------------------------

_CACHED = {}


def _build_program():
    import concourse.bacc as bacc
    import concourse.mybir as mybir
    from concourse.tile import TileContext

    f32 = mybir.dt.float32
    f16 = mybir.dt.float16
    i16 = mybir.dt.int16

    nc = bacc.Bacc("TRN2", target_bir_lowering=False, debug=False,
                   num_swdge_queues=4)

    enc = nc.dram_tensor("enc", [TABLE, BATCH], f16, kind="ExternalInput")
    idxa = nc.dram_tensor("idxa", [128, NCHUNK * CHS // 16], i16,
                          kind="ExternalInput")
    idxb = nc.dram_tensor("idxb", [128, NCHUNK * CHS // 16], i16,
                          kind="ExternalInput")
    pairs = nc.dram_tensor("pairs", [128, CHO], f16, kind="ExternalInput")
    out = nc.dram_tensor("out", [CORE_OUT, BATCH], f32, kind="ExternalOutput")

    with TileContext(nc) as tc:
        with tc.tile_pool(name="setup", bufs=1) as sp, \
             tc.tile_pool(name="gather", bufs=4) as gp, \
             tc.tile_pool(name="mid", bufs=2) as mp, \
             tc.tile_pool(name="hpsum", bufs=2, space="PSUM") as pp, \
             tc.tile_pool(name="outp", bufs=2) as outp:

            ia = sp.tile([128, NCHUNK * CHS // 16], i16, tag="ia")
            ib = sp.tile([128, NCHUNK * CHS // 16], i16, tag="ib")
            pr = sp.tile([128, CHO], f16, tag="pr")
            nc.sync.dma_start(out=ia[:, :], in_=idxa[:, :])
            nc.sync.dma_start(out=ib[:, :], in_=idxb[:, :])
            nc.sync.dma_start(out=pr[:, :], in_=pairs[:, :])

            ccols = CHS // 16        # 32 idx columns per chunk
            for k in range(NCHUNK):
                ga = gp.tile([128, 4, BATCH], f16, tag="ga")
                gb = gp.tile([128, 4, BATCH], f16, tag="gb")
                nc.gpsimd.dma_gather(
                    out_ap=ga[:, :, :], in_ap=enc[:, :],
                    idxs_ap=ia[:, k * ccols:(k + 1) * ccols],
                    num_idxs=CHS, num_idxs_reg=CHS,
                    elem_size=BATCH, queue_num=(2 * k) % 4)
                nc.gpsimd.dma_gather(
                    out_ap=gb[:, :, :], in_ap=enc[:, :],
                    idxs_ap=ib[:, k * ccols:(k + 1) * ccols],
                    num_idxs=CHS, num_idxs_reg=CHS,
                    elem_size=BATCH, queue_num=(2 * k + 1) % 4)

                h0 = mp.tile([128, 4, BATCH], f16, tag="h0")
                nc.vector.tensor_mul(h0[:, :, :], ga[:, :, :], gb[:, :, :])
                h1 = mp.tile([128, 2, BATCH], f16, tag="h1")
                nc.vector.tensor_add(
                    h1[:, :, :], h0[:, 0:2, :], h0[:, 2:4, :])
                h2 = mp.tile([128, 1, BATCH], f16, tag="h2")
                nc.vector.tensor_mul(
                    h2[:, :, :], h1[:, 0:1, :], h1[:, 1:2, :])

                # final sum pairs adjacent partitions: [128, 1024] -> [64, 1024]
                ps = pp.tile([CHO, BATCH], f32, tag="ps")
                for half in range(2):
                    nc.tensor.matmul(
                        ps[:, half * 512:(half + 1) * 512],
                        lhsT=pr[:, :],
                        rhs=h2[:, 0, half * 512:(half + 1) * 512],
                        start=True, stop=True)
                ot = outp.tile([CHO, BATCH], f32, tag="ot")
                nc.scalar.copy(ot[:, :], ps[:, :])
                nc.sync.dma_start(
                    out=out[k * CHO:(k + 1) * CHO, :], in_=ot[:, :])

    nc.compile()
    return nc


def _get_program():
    if "nc" not in _CACHED:
        _CACHED["nc"] = _build_program()
    return _CACHED["nc"]


# ----------------------------------------------------------------------------
# public entry point
# ----------------------------------------------------------------------------

def kernel(x, idx0, idx1, idx2, idx3, _trace=False, _trace_kwargs=None):
    from concourse.bass_utils import run_bass_kernel_spmd

    x = np.ascontiguousarray(np.asarray(x, dtype=np.float32))
    A, B = _compose_indices(
        np.asarray(idx0), np.asarray(idx1), np.asarray(idx2), np.asarray(idx3))

    enc = np.concatenate(
        [x, 1.0 - x,
         np.zeros((1, BATCH), np.float32),
         np.ones((1, BATCH), np.float32)], axis=0)
    enc = np.ascontiguousarray(enc.astype(np.float16))

    pairs = np.zeros((128, CHO), np.float16)
    pairs[np.arange(128), np.arange(128) // 2] = 1.0

    nc = _get_program()
    in_maps = []
    for c in range(NCORES):
        in_maps.append({"enc": enc,
                        "idxa": _core_wrap(A, c), "idxb": _core_wrap(B, c),
                        "pairs": pairs})

    kwargs = {}
    if _trace:
        kwargs["trace"] = True
        if _trace_kwargs:
            kwargs.update(_trace_kwargs)
    res = run_bass_kernel_spmd(nc, in_maps, core_ids=list(range(NCORES)), **kwargs)
    outs = [res.results[c]["out"] for c in range(NCORES)]
    full = np.concatenate(outs, axis=0)
    if _trace:
        kernel.last_exec_time_ns = res.exec_time_ns
        kernel.last_profile = res.profile_json
    return full
